# revision 8
# baseline (speedup 1.0000x reference)
"""Bass/Trainium2 kernel for nn_Net_27882927686181 (gnn_message_passing).

Computation: v0 = sigmoid(x + 1); 12 layers of
    v <- sigmoid(einsum('bmk,mk->bm', v[:, idx[l]], W[l]) + b[l])
with B=1024, M=2048, K=32, L=12.

Strategy:
  - Reformulate each layer as a dense matmul: build A_l[m, m'] with
    A_l[idx[l,m',k], m'] += W[l,m',k]  (host-side, cheap), so the layer is
        v <- sigmoid(v @ A_l + b_l).
    The PE array performs the "gather" as part of a dense GEMM instead of
    tens of thousands of scattered indirect-DMA descriptors.
  - Single NeuronCore.  Measured axon-relay behavior: single-device
    programs keep device_put inputs resident (per-call cost flat in input
    bytes), while any multi-device dispatch re-streams inputs every call
    (~0.6 ms per MB/core) and serializes per-device launches.  The dense
    kernel's HW time (1.31 ms PE stream: 512x512x12 moving rows at
    1 row/cycle @2.4 GHz, with the 100 MB f16 weight stream and all
    activations hidden under it) therefore beats any 8-way sharding.
  - Per layer: out[m', b] = sum_m A_l[m, m'] * vT[m, b] via 16 output
    tiles x 2 batch halves x 16 contraction steps of [128,128]x[128,512]
    matmuls accumulating in PSUM; ACT applies sigmoid(psum + bias)
    producing next layer's vT tiles directly in [m, b] layout.
"""

import numpy as np

B, M, K, L = 1024, 2048, 32, 12
N_CORES = 1
B_SH = B // N_CORES            # 1024 batch rows on the single core
MT = M // 128                  # 16 node tiles of 128
BH = B_SH // 512               # 2 batch halves (PSUM bank = 512 f32)

_cache = {}


def _patch_walrus():
    """Enable extra walrus passes (kept from baseline; harmless if unused)."""
    import concourse.bass_utils as bu
    if getattr(bu, "_ant_dge_patched", False):
        return
    orig = bu.run_command
    dge = ("--dge-levels=io,spill_reload,scalar_dynamic_offset,"
           "vector_dynamic_offsets,dst_reduce,transpose")

    def patched(argv, **kwargs):
        if argv and "walrus_driver" in str(argv[0]):
            argv = list(argv)
            for i, a in enumerate(argv):
                if a == "--pass":
                    passes = argv[i + 1].split(",")
                    for p in ("expand_inst_late", "coloring_allocator_reg"):
                        if p not in passes:
                            passes.insert(passes.index("codegen"), p)
                    argv[i + 1] = ",".join(passes)
                    break
            argv.append(dge)
        return orig(argv, **kwargs)

    bu.run_command = patched
    bu._ant_dge_patched = True


def _split_multi_waits(nc, max_waits=1):
    """walrus codegen rejects >max sem waits per instruction; split onto NOPs."""
    import bass_rust
    from concourse import mybir
    n = 0
    for f in nc.m.functions:
        for blk in f.blocks:
            il = blk.instructions
            i = 0
            while i < len(il):
                inst = il[i]
                si = inst.sync_info
                if si is not None and len(si.on_wait) > max_waits:
                    waits = list(si.on_wait)
                    si.on_wait = waits[:max_waits]
                    extra = waits[max_waits:]
                    pos = i
                    for j in range(0, len(extra), max_waits):
                        nop = mybir.InstNoOp(name=f"Wsplit{n}-{j}", ins=[], outs=[])
                        nop.engine = inst.engine
                        nop.sync_info = bass_rust.SyncInfo(
                            on_wait=extra[j:j + max_waits], on_update=[])
                        il.insert(pos, nop)
                        pos += 1
                        i += 1
                    n += 1
                i += 1
    return n


def _build():
    import concourse.bass as bass
    import concourse.tile as tile
    from concourse import mybir

    _patch_walrus()

    f32 = mybir.dt.float32
    f16 = mybir.dt.float16
    nc = bass.Bass("TRN2", target_bir_lowering=False, debug=False,
                   num_devices=N_CORES)

    xT_d = nc.dram_tensor("xT", [M, B_SH], f16, kind="ExternalInput").ap()
    A_d = nc.dram_tensor("Ad", [L * MT * 128, M], f16,
                         kind="ExternalInput").ap()
    b_d = nc.dram_tensor("bp", [128, L * MT], f32, kind="ExternalInput").ap()
    out_d = nc.dram_tensor("out", [M, B_SH], f16, kind="ExternalOutput").ap()

    sig = mybir.ActivationFunctionType.Sigmoid

    with tile.TileContext(nc) as tc:
        with tc.tile_pool(name="const", bufs=1) as cpool, \
             tc.tile_pool(name="A", bufs=6) as apool, \
             tc.tile_pool(name="v", bufs=2) as vpool, \
             tc.tile_pool(name="x", bufs=3) as xpool, \
             tc.tile_pool(name="o", bufs=2) as opool, \
             tc.tile_pool(name="psum", bufs=6, space="PSUM") as ppool:

            b_sb = cpool.tile([128, L * MT], f32)
            nc.sync.dma_start(b_sb[:], b_d[:])

            # ---- init: v0^T = sigmoid(xT + 1), kept in SBUF [m, b] ----
            v_cur = vpool.tile([128, MT * B_SH], f16, tag="v", name="v0")
            for t in range(MT):
                xt = xpool.tile([128, B_SH], f16, tag="x")
                nc.sync.dma_start(xt[:], xT_d[128 * t:128 * (t + 1), :])
                nc.scalar.activation(
                    v_cur[:, B_SH * t:B_SH * (t + 1)], xt[:],
                    sig, bias=1.0, scale=1.0)

            for l in range(L):
                v_next = None
                if l < L - 1:
                    v_next = vpool.tile([128, MT * B_SH], f16, tag="v",
                                        name="vnext")
                for t in range(MT):                    # output node tile m'
                    at = apool.tile([128, M], f16, tag="A")
                    nc.sync.dma_start(
                        at[:], A_d[(l * MT + t) * 128:(l * MT + t + 1) * 128, :])
                    bias_ap = b_sb[:, l * MT + t:l * MT + t + 1]
                    for h in range(BH):                # batch half (512 wide)
                        ps = ppool.tile([128, 512], f32, tag="ps")
                        for mt in range(MT):           # contraction tile m
                            nc.tensor.matmul(
                                out=ps[:],
                                lhsT=at[:, 128 * mt:128 * (mt + 1)],
                                rhs=v_cur[:, B_SH * mt + 512 * h:
                                          B_SH * mt + 512 * (h + 1)],
                                start=(mt == 0), stop=(mt == MT - 1))
                        if l < L - 1:
                            nc.scalar.activation(
                                v_next[:, B_SH * t + 512 * h:
                                       B_SH * t + 512 * (h + 1)], ps[:],
                                sig, bias=bias_ap, scale=1.0)
                        else:
                            ot = opool.tile([128, 512], f16, tag="o")
                            nc.scalar.activation(ot[:], ps[:], sig,
                                                 bias=bias_ap, scale=1.0)
                            nc.sync.dma_start(
                                out_d[128 * t:128 * (t + 1),
                                      512 * h:512 * (h + 1)], ot[:])
                v_cur = v_next

    _split_multi_waits(nc, max_waits=1)
    return nc


def _get_runner():
    if "runner" in _cache:
        return _cache["runner"]
    import jax
    import concourse.mybir as mybir
    import concourse.bass2jax as bass2jax
    from concourse.bass2jax import (_bass_exec_p, install_neuronx_cc_hook,
                                    fast_dispatch_compile)
    from jax.sharding import Mesh, PartitionSpec
    from jax.experimental.shard_map import shard_map

    nc = _build()
    install_neuronx_cc_hook()

    partition_name = nc.partition_id_tensor.name if nc.partition_id_tensor else None
    in_names, out_names, out_avals, zero_outs = [], [], [], []
    for alloc in nc.m.functions[0].allocations:
        if not isinstance(alloc, mybir.MemoryLocationSet):
            continue
        name = alloc.memorylocations[0].name
        if alloc.kind == "ExternalInput":
            if name != partition_name:
                in_names.append(name)
        elif alloc.kind == "ExternalOutput":
            shape = tuple(alloc.tensor_shape)
            dtype = mybir.dt.np(alloc.dtype)
            out_names.append(name)
            out_avals.append(jax.core.ShapedArray(shape, dtype))
            zero_outs.append(np.zeros(shape, dtype))
    n_params = len(in_names)
    all_in = in_names + out_names
    if partition_name is not None:
        all_in.append(partition_name)

    def _body(*args):
        operands = list(args)
        if partition_name is not None:
            operands.append(bass2jax.partition_id_tensor())
        return tuple(_bass_exec_p.bind(
            *operands,
            out_avals=tuple(out_avals),
            in_names=tuple(all_in),
            out_names=tuple(out_names),
            lowering_input_output_aliases=(),
            sim_require_finite=True,
            sim_require_nnan=True,
            nc=nc))

    devices = jax.devices()[:N_CORES]
    mesh = Mesh(np.asarray(devices), ("core",))
    in_specs = (PartitionSpec("core"),) * (n_params + len(out_names))
    out_specs = (PartitionSpec("core"),) * len(out_names)

    # Shapes for AOT compile (fast dispatch needs trace+compile inline).
    arg_structs = []
    for alloc in nc.m.functions[0].allocations:
        if not isinstance(alloc, mybir.MemoryLocationSet):
            continue
        name = alloc.memorylocations[0].name
        if alloc.kind == "ExternalInput" and name != partition_name:
            arg_structs.append(jax.ShapeDtypeStruct(
                (N_CORES * alloc.tensor_shape[0],) + tuple(alloc.tensor_shape[1:]),
                mybir.dt.np(alloc.dtype)))
    for av in out_avals:
        arg_structs.append(jax.ShapeDtypeStruct(
            (N_CORES * av.shape[0],) + tuple(av.shape[1:]), av.dtype))

    def mk():
        jf = jax.jit(shard_map(_body, mesh=mesh, in_specs=in_specs,
                               out_specs=out_specs, check_rep=False),
                     keep_unused=True)
        return jf.lower(*arg_structs).compile()

    try:
        f = fast_dispatch_compile(mk)
    except Exception:
        f = jax.jit(shard_map(_body, mesh=mesh, in_specs=in_specs,
                              out_specs=out_specs, check_rep=False),
                    keep_unused=True)
    _cache["runner"] = (f, in_names, out_names, zero_outs)
    return _cache["runner"]


def _prep_inputs(x, idx, W, b):
    """Host-side layout prep: densify per-layer scatter matrices."""
    # A[l][m, m'] = sum_k W[l, m', k] * [idx[l, m', k] == m]
    A = np.zeros((L, M, M), dtype=np.float32)
    cols = np.arange(M)
    for l in range(L):
        for k in range(K):
            # (idx[l,:,k], cols) pairs are unique within one statement
            # (distinct columns), so fancy += is safe; duplicates across k
            # accumulate across loop iterations.
            A[l, idx[l, :, k], cols] += W[l, :, k]
    # lhsT DRAM layout: row (l*MT + t)*128 + p, col mt*128 + j
    #   = A[l][mt*128 + p, t*128 + j]
    A_r = A.reshape(L, MT, 128, MT, 128)                 # l, mt, p, t, j
    A_p = np.ascontiguousarray(A_r.transpose(0, 3, 2, 1, 4)  # l, t, p, mt, j
                               .astype(np.float16)).reshape(L * MT * 128, M)

    # bias: bp[p, l*MT + t] = b[l, 128t + p]
    b_p = np.ascontiguousarray(
        b.astype(np.float32).reshape(L, MT, 128).transpose(2, 0, 1)
        .reshape(128, L * MT))

    xT = np.ascontiguousarray(x.T.astype(np.float16))    # [M, B]
    per_core = [{"xT": xT, "Ad": A_p, "bp": b_p}]
    return per_core


def kernel(x, idx, W, b):
    import jax
    x = np.asarray(x, dtype=np.float32)
    idx = np.asarray(idx, dtype=np.int32)
    W = np.asarray(W, dtype=np.float32)
    b = np.asarray(b, dtype=np.float32)

    f, in_names, out_names, zero_outs = _get_runner()
    per_core = _prep_inputs(x, idx, W, b)

    args = []
    for n in in_names:
        cat = np.concatenate([per_core[c][n] for c in range(N_CORES)], axis=0)
        args.append(jax.device_put(cat))
    for z in zero_outs:
        args.append(jax.device_put(np.concatenate([z] * N_CORES, axis=0)))

    outs = f(*args)
    jax.block_until_ready(outs)
    full = np.asarray(outs[0]).astype(np.float32)        # [M, B]
    return np.ascontiguousarray(full.T)                  # [B, M]


if __name__ == "__main__":
    rng = np.random.default_rng(0)
    x = rng.standard_normal((B, M)).astype(np.float32)
    idx = rng.integers(0, M, size=(L, M, K)).astype(np.int32)
    W = rng.standard_normal((L, M, K)).astype(np.float32)
    b = rng.standard_normal((L, M)).astype(np.float32)
    out = kernel(x=x, idx=idx, W=W, b=b)
    v = 1.0 / (1.0 + np.exp(-(x + 1.0)))
    for l in range(L):
        g = v[:, idx[l]]                       # [B, M, K]
        v = 1.0 / (1.0 + np.exp(-(np.einsum('bmk,mk->bm', g, W[l]) + b[l])))
    err = np.abs(out - v).max() / max(np.abs(v).max(), 1e-9)
    print("rel err vs numpy:", err)


# revision 9
# speedup vs baseline: 1.0006x; 1.0006x over previous
"""Bass/Trainium2 kernel for nn_Net_27882927686181 (gnn_message_passing).

Computation: v0 = sigmoid(x + 1); 12 layers of
    v <- sigmoid(einsum('bmk,mk->bm', v[:, idx[l]], W[l]) + b[l])
with B=1024, M=2048, K=32, L=12.

Strategy:
  - Reformulate each layer as a dense matmul: build A_l[m, m'] with
    A_l[idx[l,m',k], m'] += W[l,m',k]  (host-side, cheap), so the layer is
        v <- sigmoid(v @ A_l + b_l).
    The PE array performs the "gather" as part of a dense GEMM instead of
    tens of thousands of scattered indirect-DMA descriptors.
  - Single NeuronCore.  Measured axon-relay behavior: single-device
    programs keep device_put inputs resident (per-call cost flat in input
    bytes), while any multi-device dispatch re-streams inputs every call
    (~0.6 ms per MB/core) and serializes per-device launches.  The dense
    kernel's HW time (1.31 ms PE stream: 512x512x12 moving rows at
    1 row/cycle @2.4 GHz, with the 100 MB f16 weight stream and all
    activations hidden under it) therefore beats any 8-way sharding.
  - Per layer: out[m', b] = sum_m A_l[m, m'] * vT[m, b] via 16 output
    tiles x 2 batch halves x 16 contraction steps of [128,128]x[128,512]
    matmuls accumulating in PSUM; ACT applies sigmoid(psum + bias)
    producing next layer's vT tiles directly in [m, b] layout.
"""

import numpy as np

B, M, K, L = 1024, 2048, 32, 12
N_CORES = 1
B_SH = B // N_CORES            # 1024 batch rows on the single core
MT = M // 128                  # 16 node tiles of 128
BH = B_SH // 512               # 2 batch halves (PSUM bank = 512 f32)

_cache = {}


def _patch_walrus():
    """Enable extra walrus passes (kept from baseline; harmless if unused)."""
    import concourse.bass_utils as bu
    if getattr(bu, "_ant_dge_patched", False):
        return
    orig = bu.run_command
    dge = ("--dge-levels=io,spill_reload,scalar_dynamic_offset,"
           "vector_dynamic_offsets,dst_reduce,transpose")

    def patched(argv, **kwargs):
        if argv and "walrus_driver" in str(argv[0]):
            argv = list(argv)
            for i, a in enumerate(argv):
                if a == "--pass":
                    passes = argv[i + 1].split(",")
                    for p in ("expand_inst_late", "coloring_allocator_reg"):
                        if p not in passes:
                            passes.insert(passes.index("codegen"), p)
                    argv[i + 1] = ",".join(passes)
                    break
            argv.append(dge)
        return orig(argv, **kwargs)

    bu.run_command = patched
    bu._ant_dge_patched = True


def _split_multi_waits(nc, max_waits=1):
    """walrus codegen rejects >max sem waits per instruction; split onto NOPs."""
    import bass_rust
    from concourse import mybir
    n = 0
    for f in nc.m.functions:
        for blk in f.blocks:
            il = blk.instructions
            i = 0
            while i < len(il):
                inst = il[i]
                si = inst.sync_info
                if si is not None and len(si.on_wait) > max_waits:
                    waits = list(si.on_wait)
                    si.on_wait = waits[:max_waits]
                    extra = waits[max_waits:]
                    pos = i
                    for j in range(0, len(extra), max_waits):
                        nop = mybir.InstNoOp(name=f"Wsplit{n}-{j}", ins=[], outs=[])
                        nop.engine = inst.engine
                        nop.sync_info = bass_rust.SyncInfo(
                            on_wait=extra[j:j + max_waits], on_update=[])
                        il.insert(pos, nop)
                        pos += 1
                        i += 1
                    n += 1
                i += 1
    return n


def _build():
    import concourse.bass as bass
    import concourse.tile as tile
    from concourse import mybir

    _patch_walrus()

    f32 = mybir.dt.float32
    f16 = mybir.dt.float16
    nc = bass.Bass("TRN2", target_bir_lowering=False, debug=False,
                   num_devices=N_CORES)

    xT_d = nc.dram_tensor("xT", [M, B_SH], f16, kind="ExternalInput").ap()
    A_d = nc.dram_tensor("Ad", [L * MT * 128, M], f16,
                         kind="ExternalInput").ap()
    b_d = nc.dram_tensor("bp", [128, L * MT], f32, kind="ExternalInput").ap()
    out_d = nc.dram_tensor("out", [M, B_SH], f16, kind="ExternalOutput").ap()

    sig = mybir.ActivationFunctionType.Sigmoid

    with tile.TileContext(nc) as tc:
        with tc.tile_pool(name="const", bufs=1) as cpool, \
             tc.tile_pool(name="A", bufs=6) as apool, \
             tc.tile_pool(name="v", bufs=2) as vpool, \
             tc.tile_pool(name="o", bufs=2) as opool, \
             tc.tile_pool(name="psum", bufs=6, space="PSUM") as ppool:

            b_sb = cpool.tile([128, L * MT], f32)
            nc.sync.dma_start(b_sb[:], b_d[:])

            # ---- init: v0^T = sigmoid(xT + 1) precomputed on host; DMA
            # straight into the SBUF v-table [m, b] (no ACT on the prefix).
            v_cur = vpool.tile([128, MT * B_SH], f16, tag="v", name="v0")
            for t in range(MT):
                nc.sync.dma_start(v_cur[:, B_SH * t:B_SH * (t + 1)],
                                  xT_d[128 * t:128 * (t + 1), :])

            for l in range(L):
                v_next = None
                if l < L - 1:
                    v_next = vpool.tile([128, MT * B_SH], f16, tag="v",
                                        name="vnext")
                for t in range(MT):                    # output node tile m'
                    at = apool.tile([128, M], f16, tag="A")
                    nc.sync.dma_start(
                        at[:], A_d[(l * MT + t) * 128:(l * MT + t + 1) * 128, :])
                    bias_ap = b_sb[:, l * MT + t:l * MT + t + 1]
                    for h in range(BH):                # batch half (512 wide)
                        ps = ppool.tile([128, 512], f32, tag="ps")
                        for mt in range(MT):           # contraction tile m
                            nc.tensor.matmul(
                                out=ps[:],
                                lhsT=at[:, 128 * mt:128 * (mt + 1)],
                                rhs=v_cur[:, B_SH * mt + 512 * h:
                                          B_SH * mt + 512 * (h + 1)],
                                start=(mt == 0), stop=(mt == MT - 1))
                        if l < L - 1:
                            nc.scalar.activation(
                                v_next[:, B_SH * t + 512 * h:
                                       B_SH * t + 512 * (h + 1)], ps[:],
                                sig, bias=bias_ap, scale=1.0)
                        else:
                            ot = opool.tile([128, 512], f16, tag="o")
                            nc.scalar.activation(ot[:], ps[:], sig,
                                                 bias=bias_ap, scale=1.0)
                            nc.sync.dma_start(
                                out_d[128 * t:128 * (t + 1),
                                      512 * h:512 * (h + 1)], ot[:])
                v_cur = v_next

    _split_multi_waits(nc, max_waits=1)
    return nc


def _get_runner():
    if "runner" in _cache:
        return _cache["runner"]
    import jax
    import concourse.mybir as mybir
    import concourse.bass2jax as bass2jax
    from concourse.bass2jax import (_bass_exec_p, install_neuronx_cc_hook,
                                    fast_dispatch_compile)
    from jax.sharding import Mesh, PartitionSpec
    from jax.experimental.shard_map import shard_map

    nc = _build()
    install_neuronx_cc_hook()

    partition_name = nc.partition_id_tensor.name if nc.partition_id_tensor else None
    in_names, out_names, out_avals, zero_outs = [], [], [], []
    for alloc in nc.m.functions[0].allocations:
        if not isinstance(alloc, mybir.MemoryLocationSet):
            continue
        name = alloc.memorylocations[0].name
        if alloc.kind == "ExternalInput":
            if name != partition_name:
                in_names.append(name)
        elif alloc.kind == "ExternalOutput":
            shape = tuple(alloc.tensor_shape)
            dtype = mybir.dt.np(alloc.dtype)
            out_names.append(name)
            out_avals.append(jax.core.ShapedArray(shape, dtype))
            zero_outs.append(np.zeros(shape, dtype))
    n_params = len(in_names)
    all_in = in_names + out_names
    if partition_name is not None:
        all_in.append(partition_name)

    def _body(*args):
        operands = list(args)
        if partition_name is not None:
            operands.append(bass2jax.partition_id_tensor())
        return tuple(_bass_exec_p.bind(
            *operands,
            out_avals=tuple(out_avals),
            in_names=tuple(all_in),
            out_names=tuple(out_names),
            lowering_input_output_aliases=(),
            sim_require_finite=True,
            sim_require_nnan=True,
            nc=nc))

    devices = jax.devices()[:N_CORES]
    mesh = Mesh(np.asarray(devices), ("core",))
    in_specs = (PartitionSpec("core"),) * (n_params + len(out_names))
    out_specs = (PartitionSpec("core"),) * len(out_names)

    # Shapes for AOT compile (fast dispatch needs trace+compile inline).
    arg_structs = []
    for alloc in nc.m.functions[0].allocations:
        if not isinstance(alloc, mybir.MemoryLocationSet):
            continue
        name = alloc.memorylocations[0].name
        if alloc.kind == "ExternalInput" and name != partition_name:
            arg_structs.append(jax.ShapeDtypeStruct(
                (N_CORES * alloc.tensor_shape[0],) + tuple(alloc.tensor_shape[1:]),
                mybir.dt.np(alloc.dtype)))
    for av in out_avals:
        arg_structs.append(jax.ShapeDtypeStruct(
            (N_CORES * av.shape[0],) + tuple(av.shape[1:]), av.dtype))

    def mk():
        jf = jax.jit(shard_map(_body, mesh=mesh, in_specs=in_specs,
                               out_specs=out_specs, check_rep=False),
                     keep_unused=True)
        return jf.lower(*arg_structs).compile()

    try:
        f = fast_dispatch_compile(mk)
    except Exception:
        f = jax.jit(shard_map(_body, mesh=mesh, in_specs=in_specs,
                              out_specs=out_specs, check_rep=False),
                    keep_unused=True)
    _cache["runner"] = (f, in_names, out_names, zero_outs)
    return _cache["runner"]


def _prep_inputs(x, idx, W, b):
    """Host-side layout prep: densify per-layer scatter matrices."""
    # A[l][m, m'] = sum_k W[l, m', k] * [idx[l, m', k] == m]
    A = np.zeros((L, M, M), dtype=np.float32)
    cols = np.arange(M)
    for l in range(L):
        for k in range(K):
            # (idx[l,:,k], cols) pairs are unique within one statement
            # (distinct columns), so fancy += is safe; duplicates across k
            # accumulate across loop iterations.
            A[l, idx[l, :, k], cols] += W[l, :, k]
    # lhsT DRAM layout: row (l*MT + t)*128 + p, col mt*128 + j
    #   = A[l][mt*128 + p, t*128 + j]
    A_r = A.reshape(L, MT, 128, MT, 128)                 # l, mt, p, t, j
    A_p = np.ascontiguousarray(A_r.transpose(0, 3, 2, 1, 4)  # l, t, p, mt, j
                               .astype(np.float16)).reshape(L * MT * 128, M)

    # bias: bp[p, l*MT + t] = b[l, 128t + p]
    b_p = np.ascontiguousarray(
        b.astype(np.float32).reshape(L, MT, 128).transpose(2, 0, 1)
        .reshape(128, L * MT))

    v0 = 1.0 / (1.0 + np.exp(-(x.astype(np.float64) + 1.0)))
    xT = np.ascontiguousarray(v0.T.astype(np.float16))   # [M, B] = v0^T
    per_core = [{"xT": xT, "Ad": A_p, "bp": b_p}]
    return per_core


def kernel(x, idx, W, b):
    import jax
    x = np.asarray(x, dtype=np.float32)
    idx = np.asarray(idx, dtype=np.int32)
    W = np.asarray(W, dtype=np.float32)
    b = np.asarray(b, dtype=np.float32)

    f, in_names, out_names, zero_outs = _get_runner()
    per_core = _prep_inputs(x, idx, W, b)

    args = []
    for n in in_names:
        cat = np.concatenate([per_core[c][n] for c in range(N_CORES)], axis=0)
        args.append(jax.device_put(cat))
    for z in zero_outs:
        args.append(jax.device_put(np.concatenate([z] * N_CORES, axis=0)))

    outs = f(*args)
    jax.block_until_ready(outs)
    full = np.asarray(outs[0]).astype(np.float32)        # [M, B]
    return np.ascontiguousarray(full.T)                  # [B, M]


if __name__ == "__main__":
    rng = np.random.default_rng(0)
    x = rng.standard_normal((B, M)).astype(np.float32)
    idx = rng.integers(0, M, size=(L, M, K)).astype(np.int32)
    W = rng.standard_normal((L, M, K)).astype(np.float32)
    b = rng.standard_normal((L, M)).astype(np.float32)
    out = kernel(x=x, idx=idx, W=W, b=b)
    v = 1.0 / (1.0 + np.exp(-(x + 1.0)))
    for l in range(L):
        g = v[:, idx[l]]                       # [B, M, K]
        v = 1.0 / (1.0 + np.exp(-(np.einsum('bmk,mk->bm', g, W[l]) + b[l])))
    err = np.abs(out - v).max() / max(np.abs(v).max(), 1e-9)
    print("rel err vs numpy:", err)


# revision 11
# speedup vs baseline: 1.0241x; 1.0235x over previous
"""Bass/Trainium2 kernel for nn_Net_27882927686181 (gnn_message_passing).

Computation: v0 = sigmoid(x + 1); 12 layers of
    v <- sigmoid(einsum('bmk,mk->bm', v[:, idx[l]], W[l]) + b[l])
with B=1024, M=2048, K=32, L=12.

Strategy:
  - Reformulate each layer as a dense matmul: build A_l[m, m'] with
    A_l[idx[l,m',k], m'] += W[l,m',k]  (host-side, cheap), so the layer is
        v <- sigmoid(v @ A_l + b_l).
    The PE array performs the "gather" as part of a dense GEMM instead of
    tens of thousands of scattered indirect-DMA descriptors.
  - Single NeuronCore.  Measured axon-relay behavior: single-device
    programs keep device_put inputs resident (per-call cost flat in input
    bytes), while any multi-device dispatch re-streams inputs every call
    (~0.6 ms per MB/core) and serializes per-device launches.  The dense
    kernel's HW time (1.31 ms PE stream: 512x512x12 moving rows at
    1 row/cycle @2.4 GHz, with the 100 MB f16 weight stream and all
    activations hidden under it) therefore beats any 8-way sharding.
  - Per layer: out[m', b] = sum_m A_l[m, m'] * vT[m, b] via 16 output
    tiles x 2 batch halves x 16 contraction steps of [128,128]x[128,512]
    matmuls accumulating in PSUM; ACT applies sigmoid(psum + bias)
    producing next layer's vT tiles directly in [m, b] layout.
"""

import numpy as np

B, M, K, L = 1024, 2048, 32, 12
N_CORES = 1
B_SH = B // N_CORES            # 1024 batch rows on the single core
MT = M // 128                  # 16 node tiles of 128
BH = B_SH // 512               # 2 batch halves (PSUM bank = 512 f32)

_cache = {}


def _patch_walrus():
    """Enable extra walrus passes (kept from baseline; harmless if unused)."""
    import concourse.bass_utils as bu
    if getattr(bu, "_ant_dge_patched", False):
        return
    orig = bu.run_command
    dge = ("--dge-levels=io,spill_reload,scalar_dynamic_offset,"
           "vector_dynamic_offsets,dst_reduce,transpose")

    def patched(argv, **kwargs):
        if argv and "walrus_driver" in str(argv[0]):
            argv = list(argv)
            for i, a in enumerate(argv):
                if a == "--pass":
                    passes = argv[i + 1].split(",")
                    for p in ("expand_inst_late", "coloring_allocator_reg"):
                        if p not in passes:
                            passes.insert(passes.index("codegen"), p)
                    argv[i + 1] = ",".join(passes)
                    break
            argv.append(dge)
        return orig(argv, **kwargs)

    bu.run_command = patched
    bu._ant_dge_patched = True


def _split_multi_waits(nc, max_waits=1):
    """walrus codegen rejects >max sem waits per instruction; split onto NOPs."""
    import bass_rust
    from concourse import mybir
    n = 0
    for f in nc.m.functions:
        for blk in f.blocks:
            il = blk.instructions
            i = 0
            while i < len(il):
                inst = il[i]
                si = inst.sync_info
                if si is not None and len(si.on_wait) > max_waits:
                    waits = list(si.on_wait)
                    si.on_wait = waits[:max_waits]
                    extra = waits[max_waits:]
                    pos = i
                    for j in range(0, len(extra), max_waits):
                        nop = mybir.InstNoOp(name=f"Wsplit{n}-{j}", ins=[], outs=[])
                        nop.engine = inst.engine
                        nop.sync_info = bass_rust.SyncInfo(
                            on_wait=extra[j:j + max_waits], on_update=[])
                        il.insert(pos, nop)
                        pos += 1
                        i += 1
                    n += 1
                i += 1
    return n


def _build():
    import concourse.bass as bass
    import concourse.tile as tile
    from concourse import mybir

    _patch_walrus()

    f32 = mybir.dt.float32
    f16 = mybir.dt.float16
    nc = bass.Bass("TRN2", target_bir_lowering=False, debug=False,
                   num_devices=N_CORES)

    xT_d = nc.dram_tensor("xT", [M, B_SH], f16, kind="ExternalInput").ap()
    A_d = nc.dram_tensor("Ad", [L * MT * 128, M], f16,
                         kind="ExternalInput").ap()
    b_d = nc.dram_tensor("bp", [128, L * MT], f32, kind="ExternalInput").ap()
    out_d = nc.dram_tensor("out", [M, B_SH], f16, kind="ExternalOutput").ap()

    sig = mybir.ActivationFunctionType.Sigmoid

    with tile.TileContext(nc) as tc:
        with tc.tile_pool(name="const", bufs=1) as cpool, \
             tc.tile_pool(name="A", bufs=8) as apool, \
             tc.tile_pool(name="v", bufs=2) as vpool, \
             tc.tile_pool(name="o", bufs=4) as opool, \
             tc.tile_pool(name="psum", bufs=8, space="PSUM") as ppool:

            b_sb = cpool.tile([128, L * MT], f32)
            nc.sync.dma_start(b_sb[:], b_d[:])

            # ---- init: v0^T = sigmoid(xT + 1) precomputed on host; DMA
            # straight into the SBUF v-table [m, b] (no ACT on the prefix).
            v_cur = vpool.tile([128, MT * B_SH], f16, tag="v", name="v0")
            v0_engines = [nc.scalar, nc.gpsimd, nc.sync]
            for t in range(MT):
                # spread across three engine DMA rings so the prefix loads
                # in parallel with the A-stream on the sync ring
                v0_engines[t % 3].dma_start(
                    v_cur[:, B_SH * t:B_SH * (t + 1)],
                    xT_d[128 * t:128 * (t + 1), :])

            for l in range(L):
                v_next = None
                if l < L - 1:
                    v_next = vpool.tile([128, MT * B_SH], f16, tag="v",
                                        name="vnext")
                for t in range(MT):                    # output node tile m'
                    at = apool.tile([128, M], f16, tag="A")
                    nc.sync.dma_start(
                        at[:], A_d[(l * MT + t) * 128:(l * MT + t + 1) * 128, :])
                    bias_ap = b_sb[:, l * MT + t:l * MT + t + 1]
                    for h in range(BH):                # batch half (512 wide)
                        ps = ppool.tile([128, 512], f32, tag="ps")
                        for mt in range(MT):           # contraction tile m
                            nc.tensor.matmul(
                                out=ps[:],
                                lhsT=at[:, 128 * mt:128 * (mt + 1)],
                                rhs=v_cur[:, B_SH * mt + 512 * h:
                                          B_SH * mt + 512 * (h + 1)],
                                start=(mt == 0), stop=(mt == MT - 1))
                        if l < L - 1:
                            nc.scalar.activation(
                                v_next[:, B_SH * t + 512 * h:
                                       B_SH * t + 512 * (h + 1)], ps[:],
                                sig, bias=bias_ap, scale=1.0)
                        else:
                            ot = opool.tile([128, 512], f16, tag="o")
                            nc.scalar.activation(ot[:], ps[:], sig,
                                                 bias=bias_ap, scale=1.0)
                            nc.sync.dma_start(
                                out_d[128 * t:128 * (t + 1),
                                      512 * h:512 * (h + 1)], ot[:])
                v_cur = v_next

    _split_multi_waits(nc, max_waits=1)
    return nc


def _get_runner():
    if "runner" in _cache:
        return _cache["runner"]
    import jax
    import concourse.mybir as mybir
    import concourse.bass2jax as bass2jax
    from concourse.bass2jax import (_bass_exec_p, install_neuronx_cc_hook,
                                    fast_dispatch_compile)
    from jax.sharding import Mesh, PartitionSpec
    from jax.experimental.shard_map import shard_map

    nc = _build()
    install_neuronx_cc_hook()

    partition_name = nc.partition_id_tensor.name if nc.partition_id_tensor else None
    in_names, out_names, out_avals, zero_outs = [], [], [], []
    for alloc in nc.m.functions[0].allocations:
        if not isinstance(alloc, mybir.MemoryLocationSet):
            continue
        name = alloc.memorylocations[0].name
        if alloc.kind == "ExternalInput":
            if name != partition_name:
                in_names.append(name)
        elif alloc.kind == "ExternalOutput":
            shape = tuple(alloc.tensor_shape)
            dtype = mybir.dt.np(alloc.dtype)
            out_names.append(name)
            out_avals.append(jax.core.ShapedArray(shape, dtype))
            zero_outs.append(np.zeros(shape, dtype))
    n_params = len(in_names)
    all_in = in_names + out_names
    if partition_name is not None:
        all_in.append(partition_name)

    def _body(*args):
        operands = list(args)
        if partition_name is not None:
            operands.append(bass2jax.partition_id_tensor())
        return tuple(_bass_exec_p.bind(
            *operands,
            out_avals=tuple(out_avals),
            in_names=tuple(all_in),
            out_names=tuple(out_names),
            lowering_input_output_aliases=(),
            sim_require_finite=True,
            sim_require_nnan=True,
            nc=nc))

    devices = jax.devices()[:N_CORES]
    mesh = Mesh(np.asarray(devices), ("core",))
    in_specs = (PartitionSpec("core"),) * (n_params + len(out_names))
    out_specs = (PartitionSpec("core"),) * len(out_names)

    # Shapes for AOT compile (fast dispatch needs trace+compile inline).
    arg_structs = []
    for alloc in nc.m.functions[0].allocations:
        if not isinstance(alloc, mybir.MemoryLocationSet):
            continue
        name = alloc.memorylocations[0].name
        if alloc.kind == "ExternalInput" and name != partition_name:
            arg_structs.append(jax.ShapeDtypeStruct(
                (N_CORES * alloc.tensor_shape[0],) + tuple(alloc.tensor_shape[1:]),
                mybir.dt.np(alloc.dtype)))
    for av in out_avals:
        arg_structs.append(jax.ShapeDtypeStruct(
            (N_CORES * av.shape[0],) + tuple(av.shape[1:]), av.dtype))

    def mk():
        jf = jax.jit(shard_map(_body, mesh=mesh, in_specs=in_specs,
                               out_specs=out_specs, check_rep=False),
                     keep_unused=True)
        return jf.lower(*arg_structs).compile()

    try:
        f = fast_dispatch_compile(mk)
    except Exception:
        f = jax.jit(shard_map(_body, mesh=mesh, in_specs=in_specs,
                              out_specs=out_specs, check_rep=False),
                    keep_unused=True)
    _cache["runner"] = (f, in_names, out_names, zero_outs)
    return _cache["runner"]


def _prep_inputs(x, idx, W, b):
    """Host-side layout prep: densify per-layer scatter matrices."""
    # A[l][m, m'] = sum_k W[l, m', k] * [idx[l, m', k] == m]
    A = np.zeros((L, M, M), dtype=np.float32)
    cols = np.arange(M)
    for l in range(L):
        for k in range(K):
            # (idx[l,:,k], cols) pairs are unique within one statement
            # (distinct columns), so fancy += is safe; duplicates across k
            # accumulate across loop iterations.
            A[l, idx[l, :, k], cols] += W[l, :, k]
    # lhsT DRAM layout: row (l*MT + t)*128 + p, col mt*128 + j
    #   = A[l][mt*128 + p, t*128 + j]
    A_r = A.reshape(L, MT, 128, MT, 128)                 # l, mt, p, t, j
    A_p = np.ascontiguousarray(A_r.transpose(0, 3, 2, 1, 4)  # l, t, p, mt, j
                               .astype(np.float16)).reshape(L * MT * 128, M)

    # bias: bp[p, l*MT + t] = b[l, 128t + p]
    b_p = np.ascontiguousarray(
        b.astype(np.float32).reshape(L, MT, 128).transpose(2, 0, 1)
        .reshape(128, L * MT))

    v0 = 1.0 / (1.0 + np.exp(-(x.astype(np.float64) + 1.0)))
    xT = np.ascontiguousarray(v0.T.astype(np.float16))   # [M, B] = v0^T
    per_core = [{"xT": xT, "Ad": A_p, "bp": b_p}]
    return per_core


def kernel(x, idx, W, b):
    import jax
    x = np.asarray(x, dtype=np.float32)
    idx = np.asarray(idx, dtype=np.int32)
    W = np.asarray(W, dtype=np.float32)
    b = np.asarray(b, dtype=np.float32)

    f, in_names, out_names, zero_outs = _get_runner()
    per_core = _prep_inputs(x, idx, W, b)

    args = []
    for n in in_names:
        cat = np.concatenate([per_core[c][n] for c in range(N_CORES)], axis=0)
        args.append(jax.device_put(cat))
    for z in zero_outs:
        args.append(jax.device_put(np.concatenate([z] * N_CORES, axis=0)))

    outs = f(*args)
    jax.block_until_ready(outs)
    full = np.asarray(outs[0]).astype(np.float32)        # [M, B]
    return np.ascontiguousarray(full.T)                  # [B, M]


if __name__ == "__main__":
    rng = np.random.default_rng(0)
    x = rng.standard_normal((B, M)).astype(np.float32)
    idx = rng.integers(0, M, size=(L, M, K)).astype(np.int32)
    W = rng.standard_normal((L, M, K)).astype(np.float32)
    b = rng.standard_normal((L, M)).astype(np.float32)
    out = kernel(x=x, idx=idx, W=W, b=b)
    v = 1.0 / (1.0 + np.exp(-(x + 1.0)))
    for l in range(L):
        g = v[:, idx[l]]                       # [B, M, K]
        v = 1.0 / (1.0 + np.exp(-(np.einsum('bmk,mk->bm', g, W[l]) + b[l])))
    err = np.abs(out - v).max() / max(np.abs(v).max(), 1e-9)
    print("rel err vs numpy:", err)
